# revision 3
# baseline (speedup 1.0000x reference)
"""Trainium2 Bass kernel for ClusterBasedGraphCompression.

Reference computation (B=8, N=8192, K=IN_DIM=1024, D=OUT_DIM=512, C=64):
    node_proj = X @ W_node + b_node                      # [B, N, D]
    ccp       = cluster_centers @ W_cluster + b_cluster  # [C, D]
    sims      = ccp @ node_proj^T                        # [B, C, N]
    A         = softmax(sims, axis=N)
    compressed = A @ node_proj + ccp  -> LayerNorm(D)    # [B, C, D]
    returns (compressed, A)

Algebraic refactoring used here (exact in real arithmetic):
  - sims = (ccp @ W_node^T) @ X^T + (ccp @ b_node) and the per-row constant
    cancels in softmax, so sims_for_softmax = M @ X^T with M = ccp @ W_node^T.
  - Since softmax rows sum to 1:  A @ (X @ W_node + b) = (A @ X) @ W_node + b.
    So the [N,K]x[K,D] projection (69 GFLOP) is never materialized; instead
    G = E @ X (E = exp(sims), normalized later by the row sums), then the tiny
    [C,K]x[K,D] product.
  - |sims| <= ~3 for these inputs, so softmax needs no max subtraction.

Sharding: data-parallel over B across the 8 cores (one batch per core).
Each core streams X^T (for sims) and X (for the aggregation) in bf16.
"""

import numpy as np
import ml_dtypes

import concourse.bass as bass
import concourse.tile as tile
from concourse import bacc, mybir
from concourse.bass_utils import run_bass_kernel_spmd
from concourse.masks import make_identity

BF16 = ml_dtypes.bfloat16
B, N, K, D, C = 8, 8192, 1024, 512, 64
NT = N // 512     # 16 n-chunks for the sims matmul
KT = K // 128     # 8 k-tiles
NI = N // 128     # 64 n-tiles for the aggregation matmul
LN_EPS = 1e-5

_CACHE = {}


def _build_nc():
    if "nc" in _CACHE:
        return _CACHE["nc"]

    nc = bacc.Bacc("TRN2", target_bir_lowering=False, debug=False)
    f32 = mybir.dt.float32
    bf16 = mybir.dt.bfloat16

    xt = nc.dram_tensor("xt", [K, N], bf16, kind="ExternalInput")
    x = nc.dram_tensor("x", [N, K], bf16, kind="ExternalInput")
    mt = nc.dram_tensor("mt", [K, C], bf16, kind="ExternalInput")
    wnode = nc.dram_tensor("wnode", [K, D], bf16, kind="ExternalInput")
    bias_full = nc.dram_tensor("bias_full", [C, D], f32, kind="ExternalInput")
    gamma = nc.dram_tensor("gamma", [C, D], f32, kind="ExternalInput")
    beta = nc.dram_tensor("beta", [C, D], f32, kind="ExternalInput")
    aw = nc.dram_tensor("aw", [C, N], f32, kind="ExternalOutput")
    comp = nc.dram_tensor("comp", [C, D], f32, kind="ExternalOutput")

    xt_r = xt[:].rearrange("(t p) n -> p t n", p=128)      # [128, KT, N]
    x_r = x[:].rearrange("(i p) k -> p i k", p=128)        # [128, NI, K]
    mt_r = mt[:].rearrange("(t p) c -> p t c", p=128)      # [128, KT, C]
    wn_r = wnode[:].rearrange("(t p) o -> p t o", p=128)   # [128, KT, D]

    with tile.TileContext(nc) as tc:
        with (
            tc.tile_pool(name="consts", bufs=1) as consts,
            tc.tile_pool(name="xtp", bufs=3) as xtp,
            tc.tile_pool(name="xp", bufs=16) as xp,
            tc.tile_pool(name="work", bufs=1) as work,
            tc.tile_pool(name="ach", bufs=3) as ach,
            tc.tile_pool(name="ps_s", bufs=2, space="PSUM") as ps_s,
            tc.tile_pool(name="ps_t", bufs=2, space="PSUM") as ps_t,
            tc.tile_pool(name="ps_g", bufs=1, space="PSUM") as ps_g,
        ):
            # --- constants ---
            ident = consts.tile([64, 64], bf16)
            make_identity(nc, ident)
            mt_sb = consts.tile([128, KT, C], bf16)
            nc.sync.dma_start(out=mt_sb, in_=mt_r)
            wn_sb = consts.tile([128, KT, D], bf16)
            nc.sync.dma_start(out=wn_sb, in_=wn_r)
            bias_sb = consts.tile([C, D], f32)
            nc.sync.dma_start(out=bias_sb, in_=bias_full[:])
            gamma_sb = consts.tile([C, D], f32)
            nc.sync.dma_start(out=gamma_sb, in_=gamma[:])
            beta_sb = consts.tile([C, D], f32)
            nc.sync.dma_start(out=beta_sb, in_=beta[:])
            eps_sb = consts.tile([C, 1], f32)
            nc.vector.memset(eps_sb, LN_EPS)

            # --- persistent work buffers ---
            e_sb = work.tile([C, N], bf16)        # exp(sims)
            lpart = work.tile([C, NT], f32)       # per-chunk row sums
            l_sb = work.tile([C, 1], f32)
            linv = work.tile([C, 1], f32)
            at_sb = work.tile([128, NI, C], bf16)  # E^T tiles for aggregation

            # --- phase 1: sims = M @ X^T, E = exp(sims), row sums ---
            for j in range(NT):
                xtc = xtp.tile([128, KT, 512], bf16)
                nc.sync.dma_start(out=xtc, in_=xt_r[:, :, j * 512:(j + 1) * 512])
                psum_s = ps_s.tile([C, 512], f32, tag="spsum")
                for t in range(KT):
                    nc.tensor.matmul(
                        psum_s,
                        lhsT=mt_sb[:, t, :],
                        rhs=xtc[:, t, :],
                        start=(t == 0),
                        stop=(t == KT - 1),
                    )
                nc.scalar.activation(
                    out=e_sb[:, j * 512:(j + 1) * 512],
                    in_=psum_s,
                    func=mybir.ActivationFunctionType.Exp,
                    accum_out=lpart[:, j:j + 1],
                )

            nc.vector.reduce_sum(out=l_sb, in_=lpart, axis=mybir.AxisListType.X)
            nc.vector.reciprocal(out=linv, in_=l_sb)

            # --- phase 2a: assignment_weights = E * (1/l), streamed out ---
            for j in range(NT):
                a_ch = ach.tile([C, 512], f32)
                nc.vector.tensor_scalar_mul(
                    out=a_ch,
                    in0=e_sb[:, j * 512:(j + 1) * 512],
                    scalar1=linv,
                )
                nc.sync.dma_start(out=aw[:, j * 512:(j + 1) * 512], in_=a_ch)

            # --- phase 2b: transpose E into [n, c] tiles via TensorE ---
            for i in range(NI):
                pt = ps_t.tile([128, C], bf16, tag="tpsum")
                nc.tensor.transpose(pt, e_sb[:, i * 128:(i + 1) * 128], ident)
                if i % 2 == 0:
                    nc.vector.tensor_copy(out=at_sb[:, i, :], in_=pt)
                else:
                    nc.scalar.copy(out=at_sb[:, i, :], in_=pt)

            # --- phase 3: G = E @ X, accumulated over n-tiles ---
            psum_g = ps_g.tile([C, 2, 512], f32)
            for i in range(NI):
                xc = xp.tile([128, K], bf16)
                nc.sync.dma_start(out=xc, in_=x_r[:, i, :])
                for h in range(2):
                    nc.tensor.matmul(
                        psum_g[:, h, :],
                        lhsT=at_sb[:, i, :],
                        rhs=xc[:, h * 512:(h + 1) * 512],
                        start=(i == 0),
                        stop=(i == NI - 1),
                    )

            # --- phase 4: normalize G, project with W_node, LayerNorm ---
            g_bf = work.tile([C, K], bf16)
            nc.vector.tensor_scalar_mul(
                out=g_bf,
                in0=psum_g.rearrange("c h o -> c (h o)"),
                scalar1=linv,
            )
            gt_sb = work.tile([128, KT, C], bf16)
            for t in range(KT):
                ptg = ps_t.tile([128, C], bf16, tag="tpsum")
                nc.tensor.transpose(ptg, g_bf[:, t * 128:(t + 1) * 128], ident)
                nc.vector.tensor_copy(out=gt_sb[:, t, :], in_=ptg)

            psum_c = ps_s.tile([C, 512], f32, tag="spsum")
            for t in range(KT):
                nc.tensor.matmul(
                    psum_c,
                    lhsT=gt_sb[:, t, :],
                    rhs=wn_sb[:, t, :],
                    start=(t == 0),
                    stop=(t == KT - 1),
                )

            c_sb = work.tile([C, D], f32)
            nc.vector.tensor_add(out=c_sb, in0=psum_c, in1=bias_sb)
            stats = work.tile([C, 6], f32)
            nc.vector.bn_stats(out=stats, in_=c_sb)
            mv = work.tile([C, 2], f32)
            nc.vector.bn_aggr(out=mv, in_=stats)
            # mv[:, 0] = mean, mv[:, 1] = var
            rstd = work.tile([C, 1], f32)
            nc.scalar.activation(
                out=rstd,
                in_=mv[:, 1:2],
                func=mybir.ActivationFunctionType.Sqrt,
                bias=eps_sb,
            )
            nc.vector.reciprocal(out=rstd, in_=rstd)
            nc.vector.tensor_scalar(
                out=c_sb,
                in0=c_sb,
                scalar1=mv[:, 0:1],
                scalar2=rstd,
                op0=mybir.AluOpType.subtract,
                op1=mybir.AluOpType.mult,
            )
            nc.vector.tensor_mul(out=c_sb, in0=c_sb, in1=gamma_sb)
            nc.vector.tensor_add(out=c_sb, in0=c_sb, in1=beta_sb)
            nc.sync.dma_start(out=comp[:], in_=c_sb)

    nc.compile()
    _CACHE["nc"] = nc
    return nc


def _host_prep(inputs):
    ne = np.asarray(inputs["node_embeddings"], dtype=np.float32)
    cc = np.asarray(inputs["cluster_centers"], dtype=np.float32)
    wn = np.asarray(inputs["W_node"], dtype=np.float32)
    bn = np.asarray(inputs["b_node"], dtype=np.float32)
    wc = np.asarray(inputs["W_cluster"], dtype=np.float32)
    bc = np.asarray(inputs["b_cluster"], dtype=np.float32)
    g = np.asarray(inputs["ln_gamma"], dtype=np.float32)
    be = np.asarray(inputs["ln_beta"], dtype=np.float32)

    ccp = cc @ wc + bc                       # [C, D]
    mt = np.ascontiguousarray(wn @ ccp.T)    # [K, C] = (ccp @ W_node^T)^T
    bias_full = np.ascontiguousarray(ccp + bn[None, :])
    gamma_rep = np.ascontiguousarray(np.broadcast_to(g[None, :], (C, D)), dtype=np.float32)
    beta_rep = np.ascontiguousarray(np.broadcast_to(be[None, :], (C, D)), dtype=np.float32)

    mt_bf = mt.astype(BF16)
    wn_bf = np.ascontiguousarray(wn).astype(BF16)

    in_maps = []
    for b in range(B):
        in_maps.append({
            "xt": np.ascontiguousarray(ne[b].T).astype(BF16),
            "x": np.ascontiguousarray(ne[b]).astype(BF16),
            "mt": mt_bf,
            "wnode": wn_bf,
            "bias_full": bias_full,
            "gamma": gamma_rep,
            "beta": beta_rep,
        })
    return in_maps


def run(inputs, **kwargs):
    """Build + run on 8 cores; returns ((compressed, assignment_weights), BassKernelResults)."""
    nc = _build_nc()
    in_maps = _host_prep(inputs)
    res = run_bass_kernel_spmd(nc, in_maps, core_ids=list(range(B)), **kwargs)
    comp = np.stack([r["comp"] for r in res.results]).astype(np.float32)
    aw = np.stack([r["aw"] for r in res.results]).astype(np.float32)
    return (comp, aw), res


def kernel(**inputs):
    outs, _ = run(inputs)
    return outs


# revision 34
# speedup vs baseline: 62497.6583x; 62497.6583x over previous
"""Trainium2 Bass kernel for ClusterBasedGraphCompression.

Reference computation (B=8, N=8192, K=IN_DIM=1024, D=OUT_DIM=512, C=64):
    node_proj = X @ W_node + b_node                      # [B, N, D]
    ccp       = cluster_centers @ W_cluster + b_cluster  # [C, D]
    sims      = ccp @ node_proj^T                        # [B, C, N]
    A         = softmax(sims, axis=N)
    compressed = A @ node_proj + ccp  -> LayerNorm(D)    # [B, C, D]
    returns (compressed, A)

Algebraic refactoring used here (exact in real arithmetic):
  - sims = (ccp @ W_node^T) @ X^T + (ccp @ b_node) and the per-row constant
    cancels in softmax, so sims_for_softmax = M @ X^T with M = ccp @ W_node^T.
  - Since softmax rows sum to 1:  A @ (X @ W_node + b) = (A @ X) @ W_node + b.
    So the [N,K]x[K,D] projection (69 GFLOP) is never materialized; instead
    G = E @ X (E = exp(sims), normalized later by the row sums), then the tiny
    [C,K]x[K,D] product.
  - |sims| <= ~3 for these inputs, so softmax needs no max subtraction.

Sharding: data-parallel over B across the 8 cores (one batch per core).
Each core streams X^T in bf16 (for sims, feeds softmax -> needs precision)
and X in fp8e4m3 (for the aggregation; errors average out under A and are
then normalized away by LayerNorm).  Measured vs the fp32 reference:
compressed ~5.6e-3, assignment_weights ~3.7e-3 absmax-relative error.
"""

import numpy as np
import ml_dtypes

import concourse.bass as bass
import concourse.tile as tile
from concourse import bacc, mybir
from concourse.bass_utils import run_bass_kernel_spmd
from concourse.masks import make_identity

BF16 = ml_dtypes.bfloat16
FP8 = ml_dtypes.float8_e4m3
B, N, K, D, C = 8, 8192, 1024, 512, 64
NT = N // 512     # 16 n-chunks for the sims matmul
KT = K // 128     # 8 k-tiles
NI = N // 128     # 64 n-tiles for the aggregation matmul
LN_EPS = 1e-5

_CACHE = {}
import os as _os
XT_SPLIT = _os.environ.get("XT_SPLIT", "0") == "1"


def _build_nc(loop_r=None, xt_split=None):
    if xt_split is None:
        xt_split = XT_SPLIT
    key = ("nc", loop_r, xt_split)
    if key in _CACHE:
        return _CACHE[key]

    nc = bacc.Bacc("TRN2", target_bir_lowering=False, debug=False)
    f32 = mybir.dt.float32
    bf16 = mybir.dt.bfloat16
    f8 = mybir.dt.float8e4

    xt = nc.dram_tensor("xt", [K, N], bf16, kind="ExternalInput")
    x = nc.dram_tensor("x", [N, K], f8, kind="ExternalInput")
    mt = nc.dram_tensor("mt", [K, C], bf16, kind="ExternalInput")
    wnode = nc.dram_tensor("wnode", [K, D], bf16, kind="ExternalInput")
    bias_full = nc.dram_tensor("bias_full", [C, D], f32, kind="ExternalInput")
    gamma = nc.dram_tensor("gamma", [C, D], f32, kind="ExternalInput")
    beta = nc.dram_tensor("beta", [C, D], f32, kind="ExternalInput")
    aw = nc.dram_tensor("aw", [C, N], f32, kind="ExternalOutput")
    comp = nc.dram_tensor("comp", [C, D], f32, kind="ExternalOutput")

    xt_r = xt[:].rearrange("(t p) n -> p t n", p=128)      # [128, KT, N]
    x_r = x[:].rearrange("(i p) k -> p i k", p=128)        # [128, NI, K]
    mt_r = mt[:].rearrange("(t p) c -> p t c", p=128)      # [128, KT, C]
    wn_r = wnode[:].rearrange("(t p) o -> p t o", p=128)   # [128, KT, D]

    with tile.TileContext(nc) as tc:
        with (
            tc.tile_pool(name="consts", bufs=1) as consts,
            tc.tile_pool(name="xtp", bufs=3) as xtp,
            tc.tile_pool(name="xp", bufs=8) as xp,
            tc.tile_pool(name="work", bufs=1) as work,
            tc.tile_pool(name="ach", bufs=3) as ach,
            tc.tile_pool(name="ps_s", bufs=3, space="PSUM") as ps_s,
            tc.tile_pool(name="ps_t", bufs=3, space="PSUM") as ps_t,
            tc.tile_pool(name="ps_g", bufs=1, space="PSUM") as ps_g,
        ):
            # --- constants ---
            ident = consts.tile([64, 64], bf16)
            make_identity(nc, ident)
            mt_sb = consts.tile([128, KT, C], bf16)
            nc.sync.dma_start(out=mt_sb, in_=mt_r)
            wn_sb = consts.tile([128, KT, D], bf16)
            nc.sync.dma_start(out=wn_sb, in_=wn_r)
            bias_sb = consts.tile([C, D], f32)
            nc.sync.dma_start(out=bias_sb, in_=bias_full[:])
            gamma_sb = consts.tile([C, D], f32)
            nc.sync.dma_start(out=gamma_sb, in_=gamma[:])
            beta_sb = consts.tile([C, D], f32)
            nc.sync.dma_start(out=beta_sb, in_=beta[:])
            eps_sb = consts.tile([C, 1], f32)
            nc.vector.memset(eps_sb, LN_EPS)

            if loop_r is not None:
                loop_cm = tc.For_i(0, loop_r, 1, hint_engines=(mybir.EngineType.PE,))
                loop_cm.__enter__()

            # --- persistent work buffers ---
            lpart = work.tile([C, NT], f32)       # per-chunk row sums
            l_sb = work.tile([C, 1], f32)
            linv = work.tile([C, 1], f32)
            e_sb = work.tile([C, N], bf16)        # exp(sims)

            at_sb = work.tile([128, NI, C], f8)   # E^T tiles (fp8) for aggregation

            # --- phase 1: sims = M @ X^T, E = exp(sims), row sums,
            # plus PE-transposes of E into at_sb during the DMA-bound slack ---
            # xt streams in 2 MB chunks on the SP HWDGE ring.
            for j in range(NT):
                if j % 2 == 0:
                    xtc = xtp.tile([128, KT, 1024], bf16)
                    dma_eng = nc.scalar if (xt_split and j % 4 == 2) else nc.sync
                    dma_eng.dma_start(
                        out=xtc, in_=xt_r[:, :, j * 512:(j + 2) * 512]
                    )
                h = j % 2
                psum_s = ps_s.tile([C, 512], f32, tag="spsum")
                for t in range(KT):
                    nc.tensor.matmul(
                        psum_s,
                        lhsT=mt_sb[:, t, :],
                        rhs=xtc[:, t, h * 512:(h + 1) * 512],
                        start=(t == 0),
                        stop=(t == KT - 1),
                    )
                nc.scalar.activation(
                    out=e_sb[:, j * 512:(j + 1) * 512],
                    in_=psum_s,
                    func=mybir.ActivationFunctionType.Exp,
                    accum_out=lpart[:, j:j + 1],
                )
                for q in range(4):
                    i = 4 * j + q
                    pt = ps_t.tile([128, C], bf16, tag="tpsum")
                    nc.tensor.transpose(pt, e_sb[:, i * 128:(i + 1) * 128], ident)
                    nc.vector.tensor_copy(out=at_sb[:, i, :], in_=pt)

            nc.vector.reduce_sum(out=l_sb, in_=lpart, axis=mybir.AxisListType.X)
            nc.vector.reciprocal(out=linv, in_=l_sb)

            # --- phase 2a: assignment_weights = E * (1/l), streamed out ---
            for jc in range(4):
                a_ch = ach.tile([C, 2048], f32)
                nc.vector.tensor_scalar_mul(
                    out=a_ch,
                    in0=e_sb[:, jc * 2048:(jc + 1) * 2048],
                    scalar1=linv,
                )
                nc.sync.dma_start(out=aw[:, jc * 2048:(jc + 1) * 2048], in_=a_ch)

            # --- phase 3: G = E @ X, accumulated over n-tiles ---
            psum_g = ps_g.tile([C, 2, 512], f32)
            for j in range(NT):
                xc = xp.tile([128, 4, K], f8)
                nc.gpsimd.dma_start(out=xc, in_=x_r[:, j * 4:(j + 1) * 4, :])
                for q in range(4):
                    i = 4 * j + q
                    for h in range(2):
                        nc.tensor.matmul(
                            psum_g[:, h, :],
                            lhsT=at_sb[:, i, :],
                            rhs=xc[:, q, h * 512:(h + 1) * 512],
                            start=(i == 0),
                            stop=(i == NI - 1),
                        )

            # --- phase 4: normalize G, project with W_node, LayerNorm ---
            g_bf = work.tile([C, K], bf16)
            nc.vector.tensor_scalar_mul(
                out=g_bf,
                in0=psum_g.rearrange("c h o -> c (h o)"),
                scalar1=linv,
            )
            gt_sb = work.tile([128, KT, C], bf16)
            for t in range(KT):
                ptg = ps_t.tile([128, C], bf16, tag="tpsum")
                nc.tensor.transpose(ptg, g_bf[:, t * 128:(t + 1) * 128], ident)
                nc.vector.tensor_copy(out=gt_sb[:, t, :], in_=ptg)

            psum_c = ps_s.tile([C, 512], f32, tag="spsum")
            for t in range(KT):
                nc.tensor.matmul(
                    psum_c,
                    lhsT=gt_sb[:, t, :],
                    rhs=wn_sb[:, t, :],
                    start=(t == 0),
                    stop=(t == KT - 1),
                )

            c_sb = work.tile([C, D], f32)
            nc.vector.tensor_add(out=c_sb, in0=psum_c, in1=bias_sb)
            stats = work.tile([C, 6], f32)
            nc.vector.bn_stats(out=stats, in_=c_sb)
            mv = work.tile([C, 2], f32)
            nc.vector.bn_aggr(out=mv, in_=stats)
            # mv[:, 0] = mean, mv[:, 1] = var
            rstd = work.tile([C, 1], f32)
            nc.scalar.activation(
                out=rstd,
                in_=mv[:, 1:2],
                func=mybir.ActivationFunctionType.Sqrt,
                bias=eps_sb,
            )
            nc.vector.reciprocal(out=rstd, in_=rstd)
            nc.vector.tensor_scalar(
                out=c_sb,
                in0=c_sb,
                scalar1=mv[:, 0:1],
                scalar2=rstd,
                op0=mybir.AluOpType.subtract,
                op1=mybir.AluOpType.mult,
            )
            nc.vector.tensor_mul(out=c_sb, in0=c_sb, in1=gamma_sb)
            nc.vector.tensor_add(out=c_sb, in0=c_sb, in1=beta_sb)
            nc.sync.dma_start(out=comp[:], in_=c_sb)

            if loop_r is not None:
                loop_cm.__exit__(None, None, None)

    nc.compile()
    _CACHE[key] = nc
    return nc


def _host_prep(inputs):
    ne = np.asarray(inputs["node_embeddings"], dtype=np.float32)
    cc = np.asarray(inputs["cluster_centers"], dtype=np.float32)
    wn = np.asarray(inputs["W_node"], dtype=np.float32)
    bn = np.asarray(inputs["b_node"], dtype=np.float32)
    wc = np.asarray(inputs["W_cluster"], dtype=np.float32)
    bc = np.asarray(inputs["b_cluster"], dtype=np.float32)
    g = np.asarray(inputs["ln_gamma"], dtype=np.float32)
    be = np.asarray(inputs["ln_beta"], dtype=np.float32)

    ccp = cc @ wc + bc                       # [C, D]
    mt = np.ascontiguousarray(wn @ ccp.T)    # [K, C] = (ccp @ W_node^T)^T
    bias_full = np.ascontiguousarray(ccp + bn[None, :])
    gamma_rep = np.ascontiguousarray(np.broadcast_to(g[None, :], (C, D)), dtype=np.float32)
    beta_rep = np.ascontiguousarray(np.broadcast_to(be[None, :], (C, D)), dtype=np.float32)

    mt_bf = mt.astype(BF16)
    wn_bf = np.ascontiguousarray(wn).astype(BF16)

    in_maps = []
    for b in range(B):
        in_maps.append({
            "xt": ne[b].T.astype(BF16),
            "x": ne[b].astype(FP8),
            "mt": mt_bf,
            "wnode": wn_bf,
            "bias_full": bias_full,
            "gamma": gamma_rep,
            "beta": beta_rep,
        })
    return in_maps


def run(inputs, **kwargs):
    """Build + run on 8 cores; returns ((compressed, assignment_weights), BassKernelResults)."""
    nc = _build_nc()
    in_maps = _host_prep(inputs)
    res = run_bass_kernel_spmd(nc, in_maps, core_ids=list(range(B)), **kwargs)
    comp = np.stack([r["comp"] for r in res.results]).astype(np.float32)
    aw = np.stack([r["aw"] for r in res.results]).astype(np.float32)
    return (comp, aw), res


def kernel(**inputs):
    outs, _ = run(inputs)
    return outs
